# revision 1
# baseline (speedup 1.0000x reference)
"""Diffusion stencil kernel for Trainium2 (8 NeuronCores).

Problem: 10 iterations of x += c*(grad0(x)+grad1(x)+grad2(x)) on a
(64, 1024, 1024) fp32 volume, torch.gradient semantics (central diffs
interior, one-sided at boundaries), c = ALPHA*DT = 0.05.

Design:
- Shard axis1 (1024) across 8 cores, 128 rows each. Full inputs are
  staged per-core with a 5-row axis1 halo, so no collectives: the kernel
  runs as 2 launches of a K=5 fused-iteration program, with host-side
  resharding between launches.
- SBUF layout: partitions = (a2-block pair j) x (a0=64); free dims =
  (a1 patch 138, a2 patch 42). Two a2-blocks of 32 columns (each with a
  5-col halo) ride in the two partition halves of every tile.
- Per level: TensorE does 5 float32r matmul passes into PSUM:
  block-diag tridiagonal (axis0 gradient incl. one-sided boundary rows)
  plus 4 shifted-window identity passes (+/-a1, +/-a2, scaled c/2).
  VectorE then does ONE fused scalar_tensor_tensor per chunk:
  out = (state * 1.0) + psum -- the identity add stays exact fp32.
  ScalarE casts state -> float32r copy (matmul operands must be f32r-
  rounded). GpSimd rebuilds boundary ghost rows/cols each level
  (x[-1] := 2x[0]-x[1] makes the central diff equal the one-sided diff).
"""
import numpy as np

NUM_ITERATIONS = 10
C = 0.5 * 0.1          # ALPHA * DT
CG = C * 0.5

D0, D1, D2 = 64, 1024, 1024
NCORES = 8
SH1 = D1 // NCORES     # 128 rows of axis1 per core
K = 5                  # fused iterations per launch
S2 = 32                # a2 columns owned per block
W2 = S2 + 2 * K        # 42 patch cols
W1 = SH1 + 2 * K       # 138 patch rows
NBLK = D2 // S2        # 32 blocks
NPAIR = NBLK // 2      # 16 pairs
D2P = D2 + 2 * K       # padded a2 extent (1034)

_cache = {}


def _build_matrices():
    # T64[q, m] = weight of input a0-row q in output a0-row m (gradient only,
    # no identity), scaled by C.  One-sided at global a0 boundaries.
    t = np.zeros((64, 64), dtype=np.float32)
    for m in range(64):
        if m == 0:
            t[0, 0] = -C
            t[1, 0] = C
        elif m == 63:
            t[62, 63] = -C
            t[63, 63] = C
        else:
            t[m - 1, m] = -CG
            t[m + 1, m] = CG
    wtri = np.zeros((128, 128), dtype=np.float32)
    wtri[:64, :64] = t
    wtri[64:, 64:] = t
    wp = np.eye(128, dtype=np.float32) * CG
    wm = np.eye(128, dtype=np.float32) * -CG
    return wtri, wp, wm


def _build_program():
    import os
    import concourse.tile as tile
    from concourse import bacc, mybir

    SKIP_GHOST = os.environ.get("KV_SKIP_GHOST", "0") == "1"
    SKIP_MM = os.environ.get("KV_SKIP_MM", "0") == "1"
    SKIP_LEVELS = os.environ.get("KV_SKIP_LEVELS", "0") == "1"
    REPEAT = int(os.environ.get("KV_REPEAT", "1"))
    CHUNKCAST = os.environ.get("KV_CHUNKCAST", "0") == "1"
    PASSMAJOR = os.environ.get("KV_PASSMAJOR", "0") == "1"
    BANKDRAIN = os.environ.get("KV_BANKDRAIN", "0") == "1"

    f32 = mybir.dt.float32
    f32r = mybir.dt.float32r
    ALU = mybir.AluOpType

    nc = bacc.Bacc(None)
    xin = nc.declare_dram_parameter("xin", [NBLK, D0, W1, W2], f32, isOutput=False)
    wtri_in = nc.declare_dram_parameter("wtri", [128, 128], f32, isOutput=False)
    wp_in = nc.declare_dram_parameter("wp", [128, 128], f32, isOutput=False)
    wm_in = nc.declare_dram_parameter("wm", [128, 128], f32, isOutput=False)
    mlo_in = nc.declare_dram_parameter("mlo", [128, 1], f32, isOutput=False)
    mhi_in = nc.declare_dram_parameter("mhi", [128, 1], f32, isOutput=False)
    xout = nc.declare_dram_parameter("xout", [NBLK, D0, SH1, S2], f32, isOutput=True)

    with tile.TileContext(nc) as tc:
        with (
            tc.tile_pool(name="wpool", bufs=1) as wpool,
            tc.tile_pool(name="state", bufs=int(os.environ.get("KV_STBUFS", "5"))) as state_pool,
            tc.tile_pool(name="crp", bufs=2) as cr_pool,
            tc.tile_pool(name="gtmp", bufs=2) as gtmp_pool,
            tc.tile_pool(name="psum", bufs=(4 if BANKDRAIN else 8),
                         space="PSUM") as psum_pool,
        ):
            # --- constants: DMA in, cast weights to f32r on ACT ---
            wtri_f = wpool.tile([128, 128], f32, tag="wtri_f")
            wp_f = wpool.tile([128, 128], f32, tag="wp_f")
            wm_f = wpool.tile([128, 128], f32, tag="wm_f")
            nc.sync.dma_start(wtri_f[:], wtri_in[:])
            nc.sync.dma_start(wp_f[:], wp_in[:])
            nc.sync.dma_start(wm_f[:], wm_in[:])
            wtri = wpool.tile([128, 128], f32r, tag="wtri")
            wp = wpool.tile([128, 128], f32r, tag="wp")
            wm = wpool.tile([128, 128], f32r, tag="wm")
            nc.scalar.copy(wtri[:], wtri_f[:])
            nc.scalar.copy(wp[:], wp_f[:])
            nc.scalar.copy(wm[:], wm_f[:])
            mlo = wpool.tile([128, 1], f32, tag="mlo")
            mhi = wpool.tile([128, 1], f32, tag="mhi")
            nc.sync.dma_start(mlo[:], mlo_in[:])
            nc.sync.dma_start(mhi[:], mhi_in[:])

            for p in range(NPAIR):
                st = state_pool.tile([128, W1, W2], f32, tag="st")
                nc.sync.dma_start(st[0:64, :, :], xin[2 * p])
                nc.sync.dma_start(st[64:128, :, :], xin[2 * p + 1])

                levels = []
                if not SKIP_LEVELS:
                    for rep in range(REPEAT):
                        levels.extend(range(K))
                for t in levels:
                    rv0, rv1 = t + 1, W1 - 1 - t     # output row range
                    cv0, cv1 = t + 1, W2 - 1 - t     # output col range
                    gc0, gc1 = t, W2 - t             # ghost-row col window
                    gr0, gr1 = t, W1 - t             # ghost-col row window

                    # --- ghost rows (a1 global edges; per-core mask blend) ---
                    if not SKIP_GHOST:
                        dlo = gtmp_pool.tile([128, 1, W2], f32, tag="g0")
                        nc.vector.scalar_tensor_tensor(
                            dlo[:, :, gc0:gc1], st[:, 5:6, gc0:gc1], 2.0,
                            st[:, 6:7, gc0:gc1], op0=ALU.mult, op1=ALU.subtract)
                        elo = gtmp_pool.tile([128, 1, W2], f32, tag="g1")
                        nc.vector.scalar_tensor_tensor(
                            elo[:, :, gc0:gc1], st[:, 4:5, gc0:gc1], -1.0,
                            dlo[:, :, gc0:gc1], op0=ALU.mult, op1=ALU.add)
                        nc.vector.scalar_tensor_tensor(
                            st[:, 4:5, gc0:gc1], elo[:, :, gc0:gc1], mlo[:, 0:1],
                            st[:, 4:5, gc0:gc1], op0=ALU.mult, op1=ALU.add)
                        dhi = gtmp_pool.tile([128, 1, W2], f32, tag="g2")
                        nc.vector.scalar_tensor_tensor(
                            dhi[:, :, gc0:gc1], st[:, W1 - 6:W1 - 5, gc0:gc1], 2.0,
                            st[:, W1 - 7:W1 - 6, gc0:gc1], op0=ALU.mult, op1=ALU.subtract)
                        ehi = gtmp_pool.tile([128, 1, W2], f32, tag="g3")
                        nc.vector.scalar_tensor_tensor(
                            ehi[:, :, gc0:gc1], st[:, W1 - 5:W1 - 4, gc0:gc1], -1.0,
                            dhi[:, :, gc0:gc1], op0=ALU.mult, op1=ALU.add)
                        nc.vector.scalar_tensor_tensor(
                            st[:, W1 - 5:W1 - 4, gc0:gc1], ehi[:, :, gc0:gc1],
                            mhi[:, 0:1], st[:, W1 - 5:W1 - 4, gc0:gc1],
                            op0=ALU.mult, op1=ALU.add)
                        # --- ghost cols (a2 global edges; blocks 0/31) ---
                        if p == 0:
                            nc.vector.scalar_tensor_tensor(
                                st[0:64, gr0:gr1, 4:5], st[0:64, gr0:gr1, 5:6], 2.0,
                                st[0:64, gr0:gr1, 6:7], op0=ALU.mult, op1=ALU.subtract)
                        if p == NPAIR - 1:
                            nc.vector.scalar_tensor_tensor(
                                st[64:128, gr0:gr1, W2 - 5:W2 - 4],
                                st[64:128, gr0:gr1, W2 - 6:W2 - 5], 2.0,
                                st[64:128, gr0:gr1, W2 - 7:W2 - 6],
                                op0=ALU.mult, op1=ALU.subtract)

                    # --- cast state -> f32r for matmul consumption (ACT) ---
                    if CHUNKCAST and t > 0:
                        # chunk casts at level t-1 filled cr_next; patch the
                        # ghost rows/cols that the ghost ops just rewrote.
                        cr = cr_next
                        nc.scalar.copy(cr[:, 4:5, gc0:gc1], st[:, 4:5, gc0:gc1])
                        nc.scalar.copy(cr[:, W1 - 5:W1 - 4, gc0:gc1],
                                       st[:, W1 - 5:W1 - 4, gc0:gc1])
                        if p == 0:
                            nc.scalar.copy(cr[0:64, gr0:gr1, 4:5],
                                           st[0:64, gr0:gr1, 4:5])
                        if p == NPAIR - 1:
                            nc.scalar.copy(cr[64:128, gr0:gr1, W2 - 5:W2 - 4],
                                           st[64:128, gr0:gr1, W2 - 5:W2 - 4])
                    else:
                        cr = cr_pool.tile([128, W1, W2], f32r, tag="cr")
                        nc.scalar.copy(cr[:, gr0:gr1, gc0:gc1], st[:, gr0:gr1, gc0:gc1])
                    if CHUNKCAST and t < K - 1:
                        cr_next = cr_pool.tile([128, W1, W2], f32r, tag="cr")

                    stn = state_pool.tile([128, W1, W2], f32, tag="st")
                    ncols = cv1 - cv0
                    dr_max = 512 // ncols
                    if BANKDRAIN:
                        # pairs of equal-dr chunks share one 2-bank psum tile;
                        # ONE fused STT drains both banks.
                        r0 = rv0
                        while r0 < rv1:
                            dr = min(dr_max, rv1 - r0)
                            G = 2 if (rv1 - r0) >= 2 * dr_max else 1
                            psb = psum_pool.tile([128, 2, 512], f32, tag="psb")
                            for k in range(G):
                                rk = r0 + k * dr
                                dst = psb[:, k, 0:dr * ncols].rearrange(
                                    "p (r c) -> p r c", c=ncols)
                                nc.tensor.matmul(dst, wtri[:],
                                                 cr[:, rk:rk + dr, cv0:cv1],
                                                 start=True, stop=False)
                                nc.tensor.matmul(dst, wp[:],
                                                 cr[:, rk + 1:rk + dr + 1, cv0:cv1],
                                                 start=False, stop=False)
                                nc.tensor.matmul(dst, wm[:],
                                                 cr[:, rk - 1:rk + dr - 1, cv0:cv1],
                                                 start=False, stop=False)
                                nc.tensor.matmul(dst, wp[:],
                                                 cr[:, rk:rk + dr, cv0 + 1:cv1 + 1],
                                                 start=False, stop=False)
                                nc.tensor.matmul(dst, wm[:],
                                                 cr[:, rk:rk + dr, cv0 - 1:cv1 - 1],
                                                 start=False, stop=True)
                            if G == 2:
                                nc.vector.scalar_tensor_tensor(
                                    stn[:, r0:r0 + 2 * dr, cv0:cv1].rearrange(
                                        "p (g r) c -> p g r c", g=2),
                                    st[:, r0:r0 + 2 * dr, cv0:cv1].rearrange(
                                        "p (g r) c -> p g r c", g=2),
                                    1.0,
                                    psb[:, :, 0:dr * ncols].rearrange(
                                        "p g (r c) -> p g r c", c=ncols),
                                    op0=ALU.mult, op1=ALU.add)
                            else:
                                nc.vector.scalar_tensor_tensor(
                                    stn[:, r0:r0 + dr, cv0:cv1],
                                    st[:, r0:r0 + dr, cv0:cv1], 1.0,
                                    psb[:, 0, 0:dr * ncols].rearrange(
                                        "p (r c) -> p r c", c=ncols),
                                    op0=ALU.mult, op1=ALU.add)
                            r0 += G * dr
                        st = stn
                        continue
                    if PASSMAJOR:
                        # groups of 4 chunks; 5 weight phases over the group
                        chunks = []
                        r0 = rv0
                        while r0 < rv1:
                            chunks.append((r0, min(dr_max, rv1 - r0)))
                            r0 += chunks[-1][1]
                        for g0 in range(0, len(chunks), 4):
                            grp = chunks[g0:g0 + 4]
                            pss = []
                            for (r0, dr) in grp:
                                ps_g = psum_pool.tile([128, dr, ncols], f32,
                                                      tag="ps")
                                pss.append(ps_g)
                            passes = [
                                (wtri, 0, 0, True, False),
                                (wp, 1, 0, False, False),
                                (wm, -1, 0, False, False),
                                (wp, 0, 1, False, False),
                                (wm, 0, -1, False, True),
                            ]
                            for (w, dr_s, dc_s, st_f, sp_f) in passes:
                                for ki, (r0, dr) in enumerate(grp):
                                    nc.tensor.matmul(
                                        pss[ki][:], w[:],
                                        cr[:, r0 + dr_s:r0 + dr + dr_s,
                                           cv0 + dc_s:cv1 + dc_s],
                                        start=st_f, stop=sp_f)
                            for ki, (r0, dr) in enumerate(grp):
                                nc.vector.scalar_tensor_tensor(
                                    stn[:, r0:r0 + dr, cv0:cv1],
                                    st[:, r0:r0 + dr, cv0:cv1], 1.0, pss[ki][:],
                                    op0=ALU.mult, op1=ALU.add)
                                if CHUNKCAST and t < K - 1:
                                    nc.scalar.copy(
                                        cr_next[:, r0:r0 + dr, cv0:cv1],
                                        stn[:, r0:r0 + dr, cv0:cv1])
                        st = stn
                        continue
                    r0 = rv0
                    while r0 < rv1:
                        dr = min(dr_max, rv1 - r0)
                        if SKIP_MM:
                            nc.vector.scalar_tensor_tensor(
                                stn[:, r0:r0 + dr, cv0:cv1],
                                st[:, r0:r0 + dr, cv0:cv1], 1.0,
                                st[:, r0:r0 + dr, cv0:cv1],
                                op0=ALU.mult, op1=ALU.add)
                            r0 += dr
                            continue
                        ps = psum_pool.tile([128, dr, ncols], f32, tag="ps")
                        nc.tensor.matmul(
                            ps[:], wtri[:], cr[:, r0:r0 + dr, cv0:cv1],
                            start=True, stop=False)
                        nc.tensor.matmul(
                            ps[:], wp[:], cr[:, r0 + 1:r0 + dr + 1, cv0:cv1],
                            start=False, stop=False)
                        nc.tensor.matmul(
                            ps[:], wm[:], cr[:, r0 - 1:r0 + dr - 1, cv0:cv1],
                            start=False, stop=False)
                        nc.tensor.matmul(
                            ps[:], wp[:], cr[:, r0:r0 + dr, cv0 + 1:cv1 + 1],
                            start=False, stop=False)
                        nc.tensor.matmul(
                            ps[:], wm[:], cr[:, r0:r0 + dr, cv0 - 1:cv1 - 1],
                            start=False, stop=True)
                        nc.vector.scalar_tensor_tensor(
                            stn[:, r0:r0 + dr, cv0:cv1],
                            st[:, r0:r0 + dr, cv0:cv1], 1.0, ps[:],
                            op0=ALU.mult, op1=ALU.add)
                        if CHUNKCAST and t < K - 1:
                            nc.scalar.copy(cr_next[:, r0:r0 + dr, cv0:cv1],
                                           stn[:, r0:r0 + dr, cv0:cv1])
                        r0 += dr
                    st = stn

                nc.sync.dma_start(
                    xout[2 * p], st[0:64, K:K + SH1, K:K + S2])
                nc.sync.dma_start(
                    xout[2 * p + 1], st[64:128, K:K + SH1, K:K + S2])

    nc.finalize()
    return nc


def _stage_inputs(xfull):
    """Per-core, per-block contiguous input tiles (NBLK, D0, W1, W2)."""
    wtri, wp, wm = _cache["mats"]
    in_maps = []
    for c in range(NCORES):
        slab = np.zeros((D0, W1, D2P), dtype=np.float32)
        r0 = c * SH1 - K
        rlo = max(r0, 0)
        rhi = min(c * SH1 + SH1 + K, D1)
        slab[:, rlo - r0:rhi - r0, K:K + D2] = xfull[:, rlo:rhi, :]
        xt = np.empty((NBLK, D0, W1, W2), dtype=np.float32)
        for b in range(NBLK):
            xt[b] = slab[:, :, b * S2:b * S2 + W2]
        in_maps.append({
            "xin": xt,
            "wtri": wtri, "wp": wp, "wm": wm,
            "mlo": np.full((128, 1), 1.0 if c == 0 else 0.0, np.float32),
            "mhi": np.full((128, 1), 1.0 if c == NCORES - 1 else 0.0, np.float32),
        })
    return in_maps


def _run_pass(xfull, trace=False):
    from concourse.bass_utils import run_bass_kernel_spmd
    nc = _cache["nc"]
    res = run_bass_kernel_spmd(nc, _stage_inputs(xfull),
                               core_ids=list(range(NCORES)), trace=trace)
    # xout per core: (NBLK, D0, SH1, S2) -> (D0, SH1, D2)
    cores = [res.results[c]["xout"].transpose(1, 2, 0, 3).reshape(D0, SH1, D2)
             for c in range(NCORES)]
    out = np.concatenate(cores, axis=1)
    return out, res.exec_time_ns


def kernel(x):
    x = np.asarray(x, dtype=np.float32)
    if "nc" not in _cache:
        _cache["mats"] = _build_matrices()
        _cache["nc"] = _build_program()
    mid, t1 = _run_pass(x)
    out, t2 = _run_pass(mid)
    _cache["exec_time_ns"] = (t1 or 0) + (t2 or 0)
    return out



# revision 2
# speedup vs baseline: 5.6903x; 5.6903x over previous
"""Diffusion stencil kernel for Trainium2 (8 NeuronCores).

Problem: 10 iterations of x += c*(grad0(x)+grad1(x)+grad2(x)) on a
(64, 1024, 1024) fp32 volume, torch.gradient semantics (central diffs
interior, one-sided at boundaries), c = ALPHA*DT = 0.05.

The wall-clock of kernel() is dominated by the ~70MB/s axon tunnel, so
the design minimizes bytes shipped:
- ONE launch with all 10 iterations fused (K=10): each core gets its
  128-row axis1 shard plus a 10-row halo, fp16, a2 zero-padded by 10 on
  each side -> per-core slab (64, 148, 1044) f16 (19.8MB), 158MB total.
- Output ships back as fp16 (64, 128, 1024) per core (134MB total).
- Donated output buffers are created ON DEVICE (jitted zeros), not
  shipped. The jitted shard_map executable is cached across calls.

Device program (per core): a2 is split into 16 blocks of 64 cols; two
blocks ride in the two 64-partition halves of each (128, 148, 84) f16
state tile (partitions = block-half x a0). Per level: ghost rows/cols
rebuild the one-sided boundary diffs (x[-1] := 2x[0]-x[1]); DVE computes
E = st + CG*(shift(+a1)-shift(-a1)+shift(+a2)-shift(-a2)); TensorE adds
the a0 gradient via a single block-diag tridiagonal f16 matmul into
PSUM; DVE drains stn = E + psum per <=512-elem chunk. State stays fp16
throughout (max abs error ~1e-2 vs the 0.19 tolerance budget).
"""
import numpy as np
from concurrent.futures import ThreadPoolExecutor

NUM_ITERATIONS = 10
C = 0.5 * 0.1          # ALPHA * DT
CG = C * 0.5

D0, D1, D2 = 64, 1024, 1024
NCORES = 8
SH1 = D1 // NCORES     # 128 rows of axis1 per core
K = NUM_ITERATIONS     # all 10 iterations fused in one launch
S2 = 64                # a2 columns owned per block
W2 = S2 + 2 * K        # 84 patch cols
W1 = SH1 + 2 * K       # 148 patch rows
NBLK = D2 // S2        # 16 blocks
NPAIR = NBLK // 2      # 8 pairs
D2P = D2 + 2 * K       # padded a2 extent (1044)

_cache = {}


def _build_wtri():
    # t[q, m] = weight of input a0-row q in output a0-row m (a0 gradient
    # only, no identity), scaled by C; one-sided at global a0 boundaries.
    t = np.zeros((64, 64), dtype=np.float32)
    for m in range(64):
        if m == 0:
            t[0, 0] = -C
            t[1, 0] = C
        elif m == 63:
            t[62, 63] = -C
            t[63, 63] = C
        else:
            t[m - 1, m] = -CG
            t[m + 1, m] = CG
    wtri = np.zeros((128, 128), dtype=np.float16)
    wtri[:64, :64] = t.astype(np.float16)
    wtri[64:, 64:] = t.astype(np.float16)
    return wtri


def _build_program():
    import concourse.tile as tile
    from concourse import bacc, mybir

    f16 = mybir.dt.float16
    f32 = mybir.dt.float32
    ALU = mybir.AluOpType

    nc = bacc.Bacc(None)
    xin = nc.declare_dram_parameter("xin", [D0, W1, D2P], f16, isOutput=False)
    wtri_in = nc.declare_dram_parameter("wtri", [128, 128], f16, isOutput=False)
    mlo_in = nc.declare_dram_parameter("mlo", [128, 1], f16, isOutput=False)
    mhi_in = nc.declare_dram_parameter("mhi", [128, 1], f16, isOutput=False)
    xout = nc.declare_dram_parameter("xout", [D0, SH1, D2], f16, isOutput=True)

    with tile.TileContext(nc) as tc:
        with (
            tc.tile_pool(name="wpool", bufs=1) as wpool,
            tc.tile_pool(name="state", bufs=3) as state_pool,
            tc.tile_pool(name="tmp", bufs=1) as tmp_pool,
            tc.tile_pool(name="gtmp", bufs=2) as gtmp_pool,
            tc.tile_pool(name="psum", bufs=8, space="PSUM") as psum_pool,
        ):
            wtri = wpool.tile([128, 128], f16, tag="wtri")
            nc.sync.dma_start(wtri[:], wtri_in[:])
            mlo = wpool.tile([128, 1], f16, tag="mlo")
            mhi = wpool.tile([128, 1], f16, tag="mhi")
            nc.sync.dma_start(mlo[:], mlo_in[:])
            nc.sync.dma_start(mhi[:], mhi_in[:])

            for p in range(NPAIR):
                st = state_pool.tile([128, W1, W2], f16, tag="st")
                nc.sync.dma_start(
                    st[0:64, :, :], xin[:, :, 2 * p * S2:2 * p * S2 + W2])
                nc.sync.dma_start(
                    st[64:128, :, :],
                    xin[:, :, (2 * p + 1) * S2:(2 * p + 1) * S2 + W2])

                for t in range(K):
                    rv0, rv1 = t + 1, W1 - 1 - t     # output row range
                    cv0, cv1 = t + 1, W2 - 1 - t     # output col range
                    gc0, gc1 = t, W2 - t             # ghost-row col window
                    gr0, gr1 = t, W1 - t             # ghost-col row window

                    # --- ghost rows (a1 global edges; per-core mask blend) ---
                    dlo = gtmp_pool.tile([128, 1, W2], f16, tag="g0")
                    nc.vector.scalar_tensor_tensor(
                        dlo[:, :, gc0:gc1], st[:, K:K + 1, gc0:gc1], 2.0,
                        st[:, K + 1:K + 2, gc0:gc1],
                        op0=ALU.mult, op1=ALU.subtract)
                    elo = gtmp_pool.tile([128, 1, W2], f16, tag="g1")
                    nc.vector.scalar_tensor_tensor(
                        elo[:, :, gc0:gc1], st[:, K - 1:K, gc0:gc1], -1.0,
                        dlo[:, :, gc0:gc1], op0=ALU.mult, op1=ALU.add)
                    nc.vector.scalar_tensor_tensor(
                        st[:, K - 1:K, gc0:gc1], elo[:, :, gc0:gc1],
                        mlo[:, 0:1], st[:, K - 1:K, gc0:gc1],
                        op0=ALU.mult, op1=ALU.add)
                    dhi = gtmp_pool.tile([128, 1, W2], f16, tag="g2")
                    nc.vector.scalar_tensor_tensor(
                        dhi[:, :, gc0:gc1], st[:, W1 - K - 1:W1 - K, gc0:gc1],
                        2.0, st[:, W1 - K - 2:W1 - K - 1, gc0:gc1],
                        op0=ALU.mult, op1=ALU.subtract)
                    ehi = gtmp_pool.tile([128, 1, W2], f16, tag="g3")
                    nc.vector.scalar_tensor_tensor(
                        ehi[:, :, gc0:gc1], st[:, W1 - K:W1 - K + 1, gc0:gc1],
                        -1.0, dhi[:, :, gc0:gc1], op0=ALU.mult, op1=ALU.add)
                    nc.vector.scalar_tensor_tensor(
                        st[:, W1 - K:W1 - K + 1, gc0:gc1], ehi[:, :, gc0:gc1],
                        mhi[:, 0:1], st[:, W1 - K:W1 - K + 1, gc0:gc1],
                        op0=ALU.mult, op1=ALU.add)
                    # --- ghost cols (a2 global edges; blocks 0 / NBLK-1) ---
                    if p == 0:
                        nc.vector.scalar_tensor_tensor(
                            st[0:64, gr0:gr1, K - 1:K],
                            st[0:64, gr0:gr1, K:K + 1], 2.0,
                            st[0:64, gr0:gr1, K + 1:K + 2],
                            op0=ALU.mult, op1=ALU.subtract)
                    if p == NPAIR - 1:
                        nc.vector.scalar_tensor_tensor(
                            st[64:128, gr0:gr1, W2 - K:W2 - K + 1],
                            st[64:128, gr0:gr1, W2 - K - 1:W2 - K], 2.0,
                            st[64:128, gr0:gr1, W2 - K - 2:W2 - K - 1],
                            op0=ALU.mult, op1=ALU.subtract)

                    # --- a1/a2 shifted diffs + identity on DVE ---
                    nr, ncl = rv1 - rv0, cv1 - cv0
                    A = tmp_pool.tile([128, W1 - 2, W2 - 2], f16, tag="A")
                    nc.vector.scalar_tensor_tensor(
                        A[:, 0:nr, 0:ncl], st[:, rv0 + 1:rv1 + 1, cv0:cv1],
                        1.0, st[:, rv0 - 1:rv1 - 1, cv0:cv1],
                        op0=ALU.mult, op1=ALU.subtract)
                    B = tmp_pool.tile([128, W1 - 2, W2 - 2], f16, tag="B")
                    nc.vector.scalar_tensor_tensor(
                        B[:, 0:nr, 0:ncl], st[:, rv0:rv1, cv0 + 1:cv1 + 1],
                        1.0, st[:, rv0:rv1, cv0 - 1:cv1 - 1],
                        op0=ALU.mult, op1=ALU.subtract)
                    E = tmp_pool.tile([128, W1 - 2, W2 - 2], f16, tag="E")
                    nc.vector.scalar_tensor_tensor(
                        E[:, 0:nr, 0:ncl], A[:, 0:nr, 0:ncl], CG,
                        st[:, rv0:rv1, cv0:cv1], op0=ALU.mult, op1=ALU.add)
                    nc.vector.scalar_tensor_tensor(
                        E[:, 0:nr, 0:ncl], B[:, 0:nr, 0:ncl], CG,
                        E[:, 0:nr, 0:ncl], op0=ALU.mult, op1=ALU.add)

                    # --- a0 gradient via tridiag matmul; drain E + psum ---
                    stn = state_pool.tile([128, W1, W2], f16, tag="st")
                    dr_max = 512 // ncl
                    r0 = rv0
                    while r0 < rv1:
                        dr = min(dr_max, rv1 - r0)
                        ps = psum_pool.tile([128, dr_max, ncl], f32, tag="ps")
                        nc.tensor.matmul(
                            ps[:, 0:dr, :], wtri[:],
                            st[:, r0:r0 + dr, cv0:cv1],
                            start=True, stop=True)
                        nc.vector.scalar_tensor_tensor(
                            stn[:, r0:r0 + dr, cv0:cv1],
                            E[:, r0 - rv0:r0 - rv0 + dr, 0:ncl], 1.0,
                            ps[:, 0:dr, :], op0=ALU.mult, op1=ALU.add)
                        r0 += dr
                    st = stn

                nc.sync.dma_start(
                    xout[:, :, 2 * p * S2:(2 * p + 1) * S2],
                    st[0:64, K:K + SH1, K:K + S2])
                nc.sync.dma_start(
                    xout[:, :, (2 * p + 1) * S2:(2 * p + 2) * S2],
                    st[64:128, K:K + SH1, K:K + S2])

    nc.finalize()
    return nc


def _get_runner():
    """Build the bass program once and wrap it in a cached jitted
    shard_map callable (vendored from run_bass_via_pjrt, minus the host
    concat and the host-shipped zero output buffers)."""
    if "runner" in _cache:
        return _cache["runner"]

    import jax
    import jax.numpy as jnp
    from jax.sharding import Mesh, PartitionSpec, NamedSharding
    from jax.experimental.shard_map import shard_map
    from concourse import bass2jax, mybir

    bass2jax.install_neuronx_cc_hook()
    nc = _build_program()

    partition_name = (nc.partition_id_tensor.name
                      if nc.partition_id_tensor else None)
    in_names, out_names, out_avals = [], [], []
    for alloc in nc.m.functions[0].allocations:
        if not isinstance(alloc, mybir.MemoryLocationSet):
            continue
        name = alloc.memorylocations[0].name
        if alloc.kind == "ExternalInput":
            if name != partition_name:
                in_names.append(name)
        elif alloc.kind == "ExternalOutput":
            out_names.append(name)
            out_avals.append(jax.core.ShapedArray(
                tuple(alloc.tensor_shape), mybir.dt.np(alloc.dtype)))
    dbg_name = nc.dbg_addr.name if nc.dbg_addr is not None else None
    if nc.dbg_addr is not None and nc.dbg_callbacks:
        raise RuntimeError("dbg callbacks unsupported")
    n_params = len(in_names)
    n_outs = len(out_names)
    all_in_names = list(in_names) + list(out_names)
    if partition_name is not None:
        all_in_names.append(partition_name)

    donate = tuple(range(n_params, n_params + n_outs))

    def _body(*args):
        operands = list(args)
        if partition_name is not None:
            operands.append(bass2jax.partition_id_tensor())
        outs = bass2jax._bass_exec_p.bind(
            *operands,
            out_avals=tuple(out_avals),
            in_names=tuple(all_in_names),
            out_names=tuple(out_names),
            lowering_input_output_aliases=(),
            sim_require_finite=True,
            sim_require_nnan=True,
            nc=nc,
        )
        return tuple(outs)

    devices = jax.devices()[:NCORES]
    mesh = Mesh(np.asarray(devices), ("core",))
    sharding = NamedSharding(mesh, PartitionSpec("core"))
    in_specs = (PartitionSpec("core"),) * (n_params + n_outs)
    out_specs = (PartitionSpec("core"),) * n_outs
    sharded = jax.jit(
        shard_map(_body, mesh=mesh, in_specs=in_specs, out_specs=out_specs,
                  check_rep=False),
        donate_argnums=donate, keep_unused=True)

    def _zeros():
        return tuple(
            jnp.zeros((NCORES * a.shape[0], *a.shape[1:]), a.dtype)
            for a in out_avals)
    zeros_fn = jax.jit(_zeros, out_shardings=(sharding,) * n_outs)

    runner = {
        "nc": nc, "sharded": sharded, "zeros_fn": zeros_fn,
        "in_names": in_names, "out_names": out_names,
        "dbg_name": dbg_name, "devices": devices,
        "sharding": sharding, "mesh": mesh, "jax": jax,
    }
    _cache["runner"] = runner
    return runner


def _stage_core(x, c, devices, jax):
    """Build core c's fp16 halo slab and start its device transfer."""
    slab = np.zeros((D0, W1, D2P), dtype=np.float16)
    r0 = c * SH1 - K
    rlo = max(r0, 0)
    rhi = min(c * SH1 + SH1 + K, D1)
    slab[:, rlo - r0:rhi - r0, K:K + D2] = x[:, rlo:rhi, :]
    return jax.device_put(slab, devices[c])


def kernel(x):
    x = np.asarray(x, dtype=np.float32)
    r = _get_runner()
    jax = r["jax"]
    devices = r["devices"]
    sharding = r["sharding"]

    # stage per-core fp16 slabs and push them through the tunnel in
    # parallel with each other (shared pipe, but staging overlaps I/O)
    with ThreadPoolExecutor(NCORES) as ex:
        shards = list(ex.map(
            lambda c: _stage_core(x, c, devices, jax), range(NCORES)))
    xin_g = jax.make_array_from_single_device_arrays(
        (NCORES * D0, W1, D2P), sharding, shards)

    # small replicated params (per-core values concatenated on axis 0)
    if "wtri_g" not in _cache:
        wtri = _build_wtri()
        _cache["wtri_g"] = jax.device_put(
            np.tile(wtri, (NCORES, 1)), sharding)
        mlo = np.zeros((NCORES * 128, 1), np.float16)
        mlo[:128] = 1.0
        mhi = np.zeros((NCORES * 128, 1), np.float16)
        mhi[-128:] = 1.0
        _cache["mlo_g"] = jax.device_put(mlo, sharding)
        _cache["mhi_g"] = jax.device_put(mhi, sharding)
        if r["dbg_name"] is not None:
            _cache["dbg_g"] = jax.device_put(
                np.zeros((NCORES, 2), np.uint32), sharding)

    args = {"xin": xin_g, "wtri": _cache["wtri_g"],
            "mlo": _cache["mlo_g"], "mhi": _cache["mhi_g"]}
    if r["dbg_name"] is not None:
        args[r["dbg_name"]] = _cache["dbg_g"]
    ordered = [args[name] for name in r["in_names"]]

    zeros = r["zeros_fn"]()
    out_arrs = r["sharded"](*ordered, *zeros)

    # fetch per-core fp16 shards in parallel, widen to f32 on host
    out_g = out_arrs[0]
    shards = sorted(out_g.addressable_shards, key=lambda s: s.index[0].start)
    full = np.empty((D0, D1, D2), dtype=np.float32)

    def _fetch(i):
        s = np.asarray(shards[i].data)          # (D0, SH1, D2) f16
        full[:, i * SH1:(i + 1) * SH1, :] = s
    with ThreadPoolExecutor(NCORES) as ex:
        list(ex.map(_fetch, range(NCORES)))
    return full
